# revision 6
# baseline (speedup 1.0000x reference)
"""Additive-attention kernel for 8 TRN2 NeuronCores — fp8 DoubleRow hybrid.

reference:
    x = concat([s, h], axis=1)            # (N, 2D)
    X = tanh(x @ W.T)                     # (N, 2*DA)
    pre = (X @ v.T).T                     # (1, N)
    out = softmax(pre, axis=1)            # (1, N)

Rows (N) sharded across 8 cores (4096 each); W, v replicated. Per core
the (4096 x 2048) @ (2048 x 2048) GEMM runs 14 of 16 k-tiles in fp8
e4m3 with perf_mode=DoubleRow (2 k-tiles contracted per 512-cycle pass,
2x the bf16 MAC rate; verified 217 ns/pass on this HW) and the last 2
k-tiles in bf16, all accumulating into the same fp32 psum. bf16
operands are pre-scaled by 256 so every product carries the same 2^16
scale as the fp8 ones; the drain folds 2^-16 into the Tanh activation.

fp8 error management (the 2e-2 gate): plain e4m3 on both operands
measures 3.1e-2. Two measures bring it to ~1.6e-2:
 - error-shaped rounding (host, at quantization time): per W-row k the
   e4m3 rounding directions are chosen so sum_n v[n]*eps_W[k,n] ~= 0,
   and per x-row m so sum_k eps_x[m,k]*h[k] ~= 0 (h = W^T v restricted
   to the fp8 k-range). This cancels the mean-field first-order score
   error exactly; flips are picked by a damage/benefit greedy that
   prefers near-boundary elements, so the residual fluctuation term
   (tanh' variance weighted) is not inflated (<5% extra eps energy).
 - the last 2 k-tiles stay bf16, cutting the remaining fluctuation
   variance by 2/16.
uint8 matmul (3x lower quant error, ISA-documented with DoubleRow) was
tried and is dead on this toolchain: walrus codegen's cayman assert set
rejects non-fp8 dtypes for matmul/ldweights; e3m4 is likewise fp8-DR
excluded (s3*_dual_fp8_restrictions allows EXP4/EXP5 only).

Structure around the PE (inherited from the bf16 baseline, which ran at
the throttled ~1.95 GHz PE roofline with <2% idle):
 - warm-up matmuls on the first arrived x slice keep the HAM activity
   clock-gate open through the DMA-bound head.
 - W streams as 7 fp8 pair-tiles (4 KB/partition each) then 2 bf16
   k-tiles, in consumption order on the sync queue; a tiny gate DMA
   after pair 3 keeps early pairs prioritized on the rings. x prefetch
   and v ride the scalar/gpsimd queues behind a W-completion gate.
 - row-tiles 0 and 1 interleave over the pair stream (8 matmuls per
   arriving W tile); later tiles run pair-outer / chunk-inner so 4
   matmuls share each stationary LDWEIGHTS (redundant loads stripped
   post-build). The last tile is chunk-major with shrinking drain
   pieces to shorten the path to the collective.
 - tail: per-tile Exp with fused accumulate, partition reduce, one
   4-byte AllGather of the 8 partial sums, broadcast, scale, store.

Host-side prep: layout (transpose/concat/cast, v replication) plus the
quantization itself (scaling, shaped rounding). All O(N*K) elementwise;
the GEMM math happens on device.
"""

import numpy as np
import ml_dtypes

N, D, DA = 32768, 1024, 1024
NCORES = 8
NS = N // NCORES            # 4096 rows per core
P = 128
MT = NS // P                # 32 row-tiles per core
KIN = 2 * D                 # 2048 contraction
KT8 = 14                    # k-tiles in fp8 (7 DoubleRow pairs)
NPAIR = KT8 // 2
KTB = 2                     # k-tiles in bf16
KC = KT8 * P                # 1792 fp8 contraction columns
NOUT = 2 * DA               # 2048 out features
NCH = 512                   # psum chunk (one bank of fp32)
NCK = NOUT // NCH           # 4 chunks

SX = 32.0                   # fp8 x scale
SW = 2048.0                 # fp8 W scale
SB = 256.0                  # bf16 operand scale (SB*SB == SX*SW)
DESCALE = 1.0 / 65536.0

F8 = ml_dtypes.float8_e4m3
BF16 = ml_dtypes.bfloat16


def _build_nc():
    from concourse import bacc, mybir, tile, bass

    f32 = mybir.dt.float32
    bf16 = mybir.dt.bfloat16
    f8 = mybir.dt.float8e4
    AF = mybir.ActivationFunctionType
    ALU = mybir.AluOpType
    AX = mybir.AxisListType
    DR = mybir.MatmulPerfMode.DoubleRow

    nc = bacc.Bacc(
        "TRN2",
        target_bir_lowering=False,
        debug=False,
        num_devices=NCORES,
    )

    xh8 = nc.declare_dram_parameter("xh8", [NS, KC], f8, isOutput=False)
    xhb = nc.declare_dram_parameter("xhb", [NS, KTB * P], bf16, isOutput=False)
    w8 = nc.declare_dram_parameter("w8", [NPAIR * P, 2 * NOUT], f8,
                                   isOutput=False)
    wb = nc.declare_dram_parameter("wb", [KTB * P, NOUT], bf16,
                                   isOutput=False)
    vr = nc.declare_dram_parameter("vr", [P, NOUT], f32, isOutput=False)
    out_ext = nc.declare_dram_parameter("out", [P, MT], f32, isOutput=True)

    with tile.TileContext(nc) as tc:
        with (
            tc.tile_pool(name="wpool", bufs=1) as wpool,
            tc.tile_pool(name="xpool", bufs=4) as xpool,
            tc.tile_pool(name="tpool", bufs=3) as tpool,
            tc.tile_pool(name="spool", bufs=1) as spool,
            tc.tile_pool(name="ppool", bufs=2, space="PSUM") as ppool,
            tc.tile_pool(name="dpool", bufs=1, space="DRAM") as dpool,
        ):
            # first x k-slice for the PE warm-up, then W pair-tiles in
            # consumption order on the sync queue (x/v ride scalar).
            xm8_0 = xpool.tile([P, KT8, P], f8, name="xm8", tag="xm8")
            xmb_0 = xpool.tile([P, KTB, P], bf16, name="xmb", tag="xmb")
            nc.sync.dma_start(out=xm8_0[:, 0, :], in_=xh8[0:P, 0:P])
            w8sb = [
                wpool.tile([P, 2, NOUT], f8, name=f"w8p{t}")
                for t in range(NPAIR)
            ]
            wbsb = [
                wpool.tile([P, NOUT], bf16, name=f"wbk{i}")
                for i in range(KTB)
            ]
            # pair 0 lands as per-chunk strip pairs so the first DR
            # matmul (j=0 needs cols 0:512 of BOTH halves) starts early
            nc.sync.dma_start(out=w8sb[0][:, 0, 0:NCH], in_=w8[0:P, 0:NCH])
            nc.sync.dma_start(out=w8sb[0][:, 1, 0:NCH],
                              in_=w8[0:P, NOUT:NOUT + NCH])
            nc.scalar.dma_start(out=xm8_0[:, 1:KT8, :],
                                in_=xh8[0:P, P:KC])
            for j in range(1, NCK):
                sl = slice(j * NCH, (j + 1) * NCH)
                nc.sync.dma_start(out=w8sb[0][:, 0, sl], in_=w8[0:P, sl])
                nc.sync.dma_start(
                    out=w8sb[0][:, 1, sl],
                    in_=w8[0:P, NOUT + j * NCH:NOUT + (j + 1) * NCH],
                )
            nc.scalar.dma_start(out=xmb_0[:, :, :], in_=xhb[0:P, :])
            for t in range(1, NPAIR // 2 + 1):
                nc.sync.dma_start(
                    out=w8sb[t][:, :, :], in_=w8[t * P:(t + 1) * P, :]
                )

            # prioritize the first half of the W stream on the rings: a
            # tiny SBUF->DRAM dma stalls the sync queue until pair 3
            # lands, so later tiles only hit the rings afterwards.
            wh_gate = dpool.tile([1, 1], f8, name="wh_gate")
            nc.sync.dma_start(out=wh_gate[:, :],
                              in_=w8sb[NPAIR // 2][0:1, 0, 0:1])
            for t in range(NPAIR // 2 + 1, NPAIR):
                nc.sync.dma_start(
                    out=w8sb[t][:, :, :], in_=w8[t * P:(t + 1) * P, :]
                )
            for i in range(KTB):
                nc.sync.dma_start(
                    out=wbsb[i][:, :], in_=wb[i * P:(i + 1) * P, :]
                )

            # PE pre-warm on the first x slice: keeps the HAM activity
            # clock-gate open before real work; plain fp8 matmuls.
            pswarm = ppool.tile([P, NCH], f32, name="ps0", tag="ps0")
            for _ in range(22):
                nc.tensor.matmul(
                    pswarm[:, 0:P], lhsT=xm8_0[:, 0, :], rhs=xm8_0[:, 0, :],
                    start=True, stop=True,
                )

            def load_xm(m, eng):
                t8 = xpool.tile([P, KT8, P], f8, name="xm8", tag="xm8")
                tb = xpool.tile([P, KTB, P], bf16, name="xmb", tag="xmb")
                eng.dma_start(out=t8[:, :, :], in_=xh8[m * P:(m + 1) * P, :])
                eng.dma_start(out=tb[:, :, :], in_=xhb[m * P:(m + 1) * P, :])
                return t8, tb

            xm_pre = [(xm8_0, xmb_0), load_xm(1, nc.scalar)]

            # rendezvous the 8 cores while the weight DMAs stream in
            sync_in = dpool.tile([1, 1], f32, name="sync_in")
            sync_out = dpool.tile(
                [1, NCORES], f32, name="sync_out", addr_space="Shared"
            )
            nc.gpsimd.collective_compute(
                "AllGather",
                ALU.bypass,
                replica_groups=[list(range(NCORES))],
                ins=[sync_in.opt()],
                outs=[sync_out.opt()],
            )
            vsb = wpool.tile([P, NOUT], f32, name="vsb")
            nc.scalar.dma_start(out=vsb[:, :], in_=vr[:, :])
            # ones row for the tail's PE partition-broadcast
            ones_t = spool.tile([1, P], f32, name="ones_t")
            nc.vector.memset(ones_t[0:1, :], 1.0)

            # gate the early x prefetches behind W completion
            wgate = spool.tile([1, 1], bf16, name="wgate")
            nc.gpsimd.tensor_copy(wgate[0:1, 0:1], wbsb[KTB - 1][0:1, 0:1])

            scores = spool.tile([P, MT], f32, name="scores")
            expv = spool.tile([P, MT], f32, name="expv")
            zrow = spool.tile([P, 1], f32, name="zrow")

            def alloc_work(m):
                psums = []
                for j in range(NCK):
                    ps = ppool.tile([P, NCH], f32, name=f"ps{j}", tag=f"ps{j}")
                    psums.append(ps)
                tmt = tpool.tile([P, NOUT], f32, name="tmt", tag="tmt")
                umt = tpool.tile([P, NOUT], f32, name="umt", tag="umt")
                acc = tpool.tile([P, NCK], f32, name="acc", tag="acc")
                return psums, tmt, umt, acc

            def mm_pair(psum, x8, t, j, start):
                nc.tensor.matmul(
                    psum[:, :],
                    lhsT=x8[:, 2 * t:2 * t + 2, :],
                    rhs=w8sb[t][:, :, j * NCH:(j + 1) * NCH],
                    start=start, stop=False,
                    perf_mode=DR,
                )

            def mm_bf(psum, xb, i, j, stop):
                nc.tensor.matmul(
                    psum[:, :],
                    lhsT=xb[:, i, :],
                    rhs=wbsb[i][:, j * NCH:(j + 1) * NCH],
                    start=False, stop=stop,
                )

            def drain(m, psums, tmt, umt, acc, j):
                sl = slice(j * NCH, (j + 1) * NCH)
                nc.scalar.activation(tmt[:, sl], psums[j][:, :], AF.Tanh,
                                     scale=DESCALE)
                nc.vector.scalar_tensor_tensor(
                    out=umt[:, sl],
                    in0=tmt[:, sl],
                    scalar=1.0,
                    in1=vsb[:, sl],
                    op0=ALU.mult,
                    op1=ALU.mult,
                    accum_out=acc[:, j:j + 1],
                )

            def finish_scores(m, acc):
                nc.vector.tensor_reduce(
                    scores[:, m:m + 1], acc[:, :], AX.X, ALU.add
                )
                nc.scalar.activation(
                    expv[:, m:m + 1], scores[:, m:m + 1], AF.Exp
                )

            # tiles 0 and 1 interleaved over the W stream: 8 matmuls per
            # arriving pair-tile keep the PE saturated while W lands
            work01 = [alloc_work(0), alloc_work(1)]
            for t in range(NPAIR):
                for m in (0, 1):
                    for j in range(NCK):
                        mm_pair(work01[m][0][j], xm_pre[m][0], t, j,
                                start=(t == 0))
            for i in range(KTB):
                for m in (0, 1):
                    for j in range(NCK):
                        mm_bf(work01[m][0][j], xm_pre[m][1], i, j,
                              stop=(i == KTB - 1))
            for m in (0, 1):
                psums, tmt, umt, acc = work01[m]
                for j in range(NCK):
                    drain(m, psums, tmt, umt, acc, j)
                finish_scores(m, acc)

            for m in range(2, MT):
                if m < 10:
                    eng = nc.gpsimd
                else:
                    eng = nc.sync if m % 2 == 0 else nc.gpsimd
                xm8, xmb = load_xm(m, eng)
                psums, tmt, umt, acc = alloc_work(m)

                if m < MT - 1:
                    for t in range(NPAIR):
                        for j in range(NCK):
                            mm_pair(psums[j], xm8, t, j, start=(t == 0))
                    for i in range(KTB):
                        for j in range(NCK):
                            mm_bf(psums[j], xmb, i, j, stop=(i == KTB - 1))
                    for j in range(NCK):
                        drain(m, psums, tmt, umt, acc, j)
                    finish_scores(m, acc)
                else:
                    # last tile: chunk-major so each chunk drains while
                    # the next chunk's matmuls run, shrinking pieces
                    acc10 = tpool.tile(
                        [P, 2 * NCK + 1], f32, name="acc10", tag="acc10"
                    )
                    NH = NCH // 2
                    NQ = NCH // 4
                    ac = 0
                    for j in range(NCK):
                        for t in range(NPAIR):
                            mm_pair(psums[j], xm8, t, j, start=(t == 0))
                        for i in range(KTB):
                            mm_bf(psums[j], xmb, i, j, stop=(i == KTB - 1))
                        widths = [NH, NH] if j < NCK - 1 else [NH, NQ, NQ]
                        off = 0
                        for w in widths:
                            sl = slice(j * NCH + off, j * NCH + off + w)
                            psl = slice(off, off + w)
                            nc.scalar.activation(
                                tmt[:, sl], psums[j][:, psl], AF.Tanh,
                                scale=DESCALE,
                            )
                            nc.vector.scalar_tensor_tensor(
                                out=umt[:, sl],
                                in0=tmt[:, sl],
                                scalar=1.0,
                                in1=vsb[:, sl],
                                op0=ALU.mult,
                                op1=ALU.mult,
                                accum_out=acc10[:, ac:ac + 1],
                            )
                            off += w
                            ac += 1
                    nc.vector.tensor_reduce(
                        scores[:, m:m + 1], acc10[:, :], AX.X, ALU.add
                    )
                    nc.scalar.activation(
                        expv[:, m:m + 1], scores[:, m:m + 1], AF.Exp
                    )

            # ---- softmax over the global N via one AllGather ----
            from concourse import bass_isa

            nc.vector.tensor_reduce(
                zrow[:, 0:1], expv[:, :], AX.X, ALU.add
            )
            zloc = spool.tile([P, 1], f32, name="zloc")
            nc.gpsimd.partition_all_reduce(
                zloc[:, 0:1], zrow[:, 0:1], channels=P,
                reduce_op=bass_isa.ReduceOp.add,
            )
            zin = dpool.tile([1, 1], f32, name="zin")
            zout = dpool.tile(
                [1, NCORES], f32, name="zout", addr_space="Shared"
            )
            nc.gpsimd.dma_start(out=zin[:, :], in_=zloc[0:1, 0:1])
            nc.gpsimd.collective_compute(
                "AllGather",
                ALU.bypass,
                replica_groups=[list(range(NCORES))],
                ins=[zin.opt()],
                outs=[zout.opt()],
            )
            # read the 8 partials into one partition, reduce, then
            # broadcast to 128 partitions with a 1-row PE ones-matmul
            # (the stride-0 DRAM broadcast read cost ~6us in gpsimd
            # dge+drain time)
            zgb = spool.tile([1, NCORES], f32, name="zgb")
            nc.sync.dma_start(out=zgb[0:1, :], in_=zout[0:1, :])
            zs = spool.tile([1, 1], f32, name="zs")
            nc.vector.tensor_reduce(zs[0:1, 0:1], zgb[0:1, :], AX.X, ALU.add)
            psb = ppool.tile([P, NCH], f32, name="ps0", tag="ps0")
            nc.tensor.matmul(
                psb[:, 0:1], lhsT=ones_t[0:1, :], rhs=zs[0:1, 0:1],
                start=True, stop=True,
            )
            rzb = spool.tile([P, 1], f32, name="rzb")
            nc.vector.reciprocal(rzb[:, 0:1], psb[:, 0:1])
            outsb = spool.tile([P, MT], f32, name="outsb")
            MH = MT // 2
            nc.vector.tensor_scalar_mul(
                outsb[:, 0:MH], expv[:, 0:MH], rzb[:, 0:1]
            )
            nc.sync.dma_start(out=out_ext[:, 0:MH], in_=outsb[:, 0:MH])
            nc.vector.tensor_scalar_mul(
                outsb[:, MH:MT], expv[:, MH:MT], rzb[:, 0:1]
            )
            nc.scalar.dma_start(out=out_ext[:, MH:MT], in_=outsb[:, MH:MT])

    nc.finalize()
    _strip_redundant_ldweights(nc)
    return nc


def _strip_redundant_ldweights(nc):
    """Bacc emits one InstLdweights per matmul even when consecutive
    matmuls share the stationary operand. Drop the redundant ones."""
    def sig(arg):
        return (
            getattr(arg, "memref", None),
            getattr(arg, "offset", None),
            str(getattr(arg, "ap", None)),
        )

    removed = 0
    for bb in nc.main_func.blocks:
        keep = []
        last = None
        for inst in bb.instructions:
            if "Ldweights" in type(inst).__name__:
                s = sig(inst.ins[0])
                si = inst.sync_info
                if s == last and (
                    si is None or (not si.on_wait and not si.on_update)
                ):
                    removed += 1
                    continue
                last = s
            keep.append(inst)
        bb.instructions = keep
    return removed


# ---------------- host-side quantization ----------------

def _shaped_round(A, wcol, nscan):
    """e4m3 rounding of A (already scaled) with per-row flip choices so
    sum_j wcol[j]*eps[i,j] ~= 0, flips picked by damage/benefit greedy
    (near-boundary elements first) to keep eps energy ~unchanged.

    The alternative rounding (fp8 neighbor on the other side of A) is
    computed with sign-magnitude bit arithmetic on the e4m3 encoding."""
    A = np.clip(np.asarray(A, dtype=np.float32), -240.0, 240.0)
    q = A.astype(F8)
    qf = q.astype(np.float32)
    eps = qf - A
    u = q.view(np.uint8)
    sgn = u & np.uint8(0x80)
    mag = u & np.uint8(0x7F)
    pos = sgn == 0
    toward_pos = eps < 0          # q < A: the other neighbor is above q
    away = toward_pos == pos      # step increases |value|
    newmag = np.where(away, mag + np.uint8(1), mag - np.uint8(1))
    newsgn = sgn.copy()
    cross = (mag == 0) & ~away    # +/-0 stepping across zero
    newsgn = np.where(cross, sgn ^ np.uint8(0x80), newsgn)
    newmag = np.where(cross, np.uint8(1), newmag)
    alt = (newsgn | newmag).astype(np.uint8).view(F8).astype(np.float32)
    alt = np.where(eps == 0, qf, alt)
    epsa = alt - A
    w32 = wcol.astype(np.float32)
    c = (epsa - eps) * w32
    damage = (w32 ** 2) * (epsa ** 2 - eps ** 2)
    with np.errstate(divide="ignore", invalid="ignore"):
        ratio = np.where(np.abs(c) > 0, damage / np.abs(c), np.inf)
    nscan = min(nscan, A.shape[1] - 1)
    part = np.argpartition(ratio, nscan, axis=1)[:, :nscan]
    subr = np.take_along_axis(ratio, part, axis=1)
    subo = np.argsort(subr, axis=1)
    order = np.take_along_axis(part, subo, axis=1)
    c_s = np.take_along_axis(c, order, axis=1)
    g = (eps.astype(np.float64) @ wcol.astype(np.float64)).astype(np.float32)
    t = -g.copy()
    flip_s = np.zeros((A.shape[0], nscan), dtype=bool)
    for _ in range(2):
        for step in range(nscan):
            cn = np.where(flip_s[:, step], 0.0, c_s[:, step])
            take = np.abs(t - cn) < np.abs(t)
            t += np.where(take, -cn, 0.0)
            flip_s[:, step] |= take
    flip = np.zeros(A.shape, dtype=bool)
    np.put_along_axis(flip, order, flip_s, axis=1)
    return np.where(flip, alt, q).astype(F8)


def _prep_core_inputs(s, h, W, v):
    """Quantize + lay out per-core inputs."""
    x = np.concatenate([s, h], axis=1)                   # [N, KIN] f32
    Wt = np.ascontiguousarray(W.T)                       # [KIN, NOUT]
    vv = v.reshape(-1).astype(np.float64)
    hvec = Wt.astype(np.float64) @ vv                    # [KIN]

    W8 = _shaped_round(Wt[:KC].astype(np.float64) * SW, vv, nscan=512)
    X8 = _shaped_round(x[:, :KC].astype(np.float64) * SX, hvec[:KC],
                       nscan=384)

    # W pair tiles: w8[t*128+kk, i*NOUT+n] = W8[(2t+i)*128+kk, n]
    w8v = (
        W8.reshape(NPAIR, 2, P, NOUT)
        .transpose(0, 2, 1, 3)
        .reshape(NPAIR * P, 2 * NOUT)
    )
    w8v = np.ascontiguousarray(w8v)
    wbv = np.ascontiguousarray(Wt[KC:] * SB).astype(BF16)   # [256, NOUT]
    vrep = np.ascontiguousarray(
        np.broadcast_to(v.reshape(1, NOUT), (P, NOUT))
    ).astype(np.float32)

    xbv = (x[:, KC:] * SB).astype(BF16)                     # [N, 256]

    in_maps = []
    for c in range(NCORES):
        sl = slice(c * NS, (c + 1) * NS)
        x8c = X8[sl]                                        # [NS, KC]
        xh8 = (
            x8c.reshape(MT, P, KT8, P)
            .transpose(0, 3, 2, 1)
            .reshape(NS, KC)
        )
        xh8 = np.ascontiguousarray(xh8)
        xbc = xbv[sl]
        xhb = (
            xbc.reshape(MT, P, KTB, P)
            .transpose(0, 3, 2, 1)
            .reshape(NS, KTB * P)
        )
        xhb = np.ascontiguousarray(xhb)
        in_maps.append(
            {"xh8": xh8, "xhb": xhb, "w8": w8v, "wb": wbv, "vr": vrep}
        )
    return in_maps


_RUN_KW = {}  # test.py can inject trace=True etc.
LAST_RESULT = None


def kernel(s, h, W, v):
    from concourse.bass_utils import run_bass_kernel_spmd

    global LAST_RESULT
    s = np.asarray(s, dtype=np.float32)
    h = np.asarray(h, dtype=np.float32)
    W = np.asarray(W, dtype=np.float32)
    v = np.asarray(v, dtype=np.float32)

    in_maps = _prep_core_inputs(s, h, W, v)
    res = None
    for attempt in range(3):
        nc = _build_nc()
        try:
            res = run_bass_kernel_spmd(
                nc, in_maps, core_ids=list(range(NCORES)), **_RUN_KW
            )
            break
        except Exception:
            if attempt == 2:
                raise
            import time
            time.sleep(15)
    LAST_RESULT = res

    outs = []
    for c in range(NCORES):
        oc = np.asarray(res.results[c]["out"], dtype=np.float32)  # [P, MT]
        outs.append(oc.T.reshape(-1))
    return np.concatenate(outs).reshape(1, N).astype(np.float32)


# revision 7
# speedup vs baseline: 1.1335x; 1.1335x over previous
"""Additive-attention kernel for 8 TRN2 NeuronCores — fp8 DoubleRow hybrid.

reference:
    x = concat([s, h], axis=1)            # (N, 2D)
    X = tanh(x @ W.T)                     # (N, 2*DA)
    pre = (X @ v.T).T                     # (1, N)
    out = softmax(pre, axis=1)            # (1, N)

Rows (N) sharded across 8 cores (4096 each); W, v replicated. Per core
the (4096 x 2048) @ (2048 x 2048) GEMM runs 14 of 16 k-tiles in fp8
e4m3 with perf_mode=DoubleRow (2 k-tiles contracted per 512-cycle pass,
2x the bf16 MAC rate; verified 217 ns/pass on this HW) and the last 2
k-tiles in bf16, all accumulating into the same fp32 psum. bf16
operands are pre-scaled by 256 so every product carries the same 2^16
scale as the fp8 ones; the drain folds 2^-16 into the Tanh activation.

fp8 error management (the 2e-2 gate): plain e4m3 on both operands
measures 3.1e-2. Two measures bring it to ~1.6e-2:
 - error-shaped rounding (host, at quantization time): per W-row k the
   e4m3 rounding directions are chosen so sum_n v[n]*eps_W[k,n] ~= 0,
   and per x-row m so sum_k eps_x[m,k]*h[k] ~= 0 (h = W^T v restricted
   to the fp8 k-range). This cancels the mean-field first-order score
   error exactly; flips are picked by a damage/benefit greedy that
   prefers near-boundary elements, so the residual fluctuation term
   (tanh' variance weighted) is not inflated (<5% extra eps energy).
 - the last 2 k-tiles stay bf16, cutting the remaining fluctuation
   variance by 2/16.
uint8 matmul (3x lower quant error, ISA-documented with DoubleRow) was
tried and is dead on this toolchain: walrus codegen's cayman assert set
rejects non-fp8 dtypes for matmul/ldweights; e3m4 is likewise fp8-DR
excluded (s3*_dual_fp8_restrictions allows EXP4/EXP5 only).

Structure around the PE (inherited from the bf16 baseline, which ran at
the throttled ~1.95 GHz PE roofline with <2% idle):
 - warm-up matmuls on the first arrived x slice keep the HAM activity
   clock-gate open through the DMA-bound head.
 - W streams as 7 fp8 pair-tiles (4 KB/partition each) then 2 bf16
   k-tiles, in consumption order on the sync queue; a tiny gate DMA
   after pair 3 keeps early pairs prioritized on the rings. x prefetch
   and v ride the scalar/gpsimd queues behind a W-completion gate.
 - row-tiles 0 and 1 interleave over the pair stream (8 matmuls per
   arriving W tile); later tiles run pair-outer / chunk-inner so 4
   matmuls share each stationary LDWEIGHTS (redundant loads stripped
   post-build). The last tile is chunk-major with shrinking drain
   pieces to shorten the path to the collective.
 - tail: per-tile Exp with fused accumulate, partition reduce, one
   4-byte AllGather of the 8 partial sums, broadcast, scale, store.

Host-side prep: layout (transpose/concat/cast, v replication) plus the
quantization itself (scaling, shaped rounding). All O(N*K) elementwise;
the GEMM math happens on device.
"""

import numpy as np
import ml_dtypes

N, D, DA = 32768, 1024, 1024
NCORES = 8
NS = N // NCORES            # 4096 rows per core
P = 128
MT = NS // P                # 32 row-tiles per core
KIN = 2 * D                 # 2048 contraction
KT8 = 16                    # k-tiles in fp8 (8 DoubleRow pairs)
NPAIR = KT8 // 2
KTB = 16 - KT8              # k-tiles in bf16 (0 = all-fp8)
KC = KT8 * P                # 1792 fp8 contraction columns
NOUT = 2 * DA               # 2048 out features
NCH = 512                   # psum chunk (one bank of fp32)
NCK = NOUT // NCH           # 4 chunks

SX = 32.0                   # fp8 x scale
SW = 2048.0                 # fp8 W scale
SB = 256.0                  # bf16 operand scale (SB*SB == SX*SW)
DESCALE = 1.0 / 65536.0

F8 = ml_dtypes.float8_e4m3
BF16 = ml_dtypes.bfloat16


def _build_nc():
    from concourse import bacc, mybir, tile, bass

    f32 = mybir.dt.float32
    bf16 = mybir.dt.bfloat16
    f8 = mybir.dt.float8e4
    AF = mybir.ActivationFunctionType
    ALU = mybir.AluOpType
    AX = mybir.AxisListType
    DR = mybir.MatmulPerfMode.DoubleRow

    nc = bacc.Bacc(
        "TRN2",
        target_bir_lowering=False,
        debug=False,
        num_devices=NCORES,
    )

    xh8 = nc.declare_dram_parameter("xh8", [NS, KC], f8, isOutput=False)
    w8 = nc.declare_dram_parameter("w8", [NPAIR * P, 2 * NOUT], f8,
                                   isOutput=False)
    if KTB:
        xhb = nc.declare_dram_parameter("xhb", [NS, KTB * P], bf16,
                                        isOutput=False)
        wb = nc.declare_dram_parameter("wb", [KTB * P, NOUT], bf16,
                                       isOutput=False)
    vr = nc.declare_dram_parameter("vr", [P, NOUT], f32, isOutput=False)
    out_ext = nc.declare_dram_parameter("out", [P, MT], f32, isOutput=True)

    with tile.TileContext(nc) as tc:
        with (
            tc.tile_pool(name="wpool", bufs=1) as wpool,
            tc.tile_pool(name="xpool", bufs=4) as xpool,
            tc.tile_pool(name="tpool", bufs=3) as tpool,
            tc.tile_pool(name="spool", bufs=1) as spool,
            tc.tile_pool(name="ppool", bufs=2, space="PSUM") as ppool,
            tc.tile_pool(name="dpool", bufs=1, space="DRAM") as dpool,
        ):
            # first x k-slice for the PE warm-up, then W pair-tiles in
            # consumption order on the sync queue (x/v ride scalar).
            xm8_0 = xpool.tile([P, KT8, P], f8, name="xm8", tag="xm8")
            xmb_0 = (xpool.tile([P, KTB, P], bf16, name="xmb", tag="xmb")
                     if KTB else None)
            nc.sync.dma_start(out=xm8_0[:, 0, :], in_=xh8[0:P, 0:P])
            w8sb = [
                wpool.tile([P, 2, NOUT], f8, name=f"w8p{t}")
                for t in range(NPAIR)
            ]
            wbsb = [
                wpool.tile([P, NOUT], bf16, name=f"wbk{i}")
                for i in range(KTB)
            ]
            # pair 0 lands as per-chunk strip pairs so the first DR
            # matmul (j=0 needs cols 0:512 of BOTH halves) starts early
            nc.sync.dma_start(out=w8sb[0][:, 0, 0:NCH], in_=w8[0:P, 0:NCH])
            nc.sync.dma_start(out=w8sb[0][:, 1, 0:NCH],
                              in_=w8[0:P, NOUT:NOUT + NCH])
            nc.scalar.dma_start(out=xm8_0[:, 1:KT8, :],
                                in_=xh8[0:P, P:KC])
            for j in range(1, NCK):
                sl = slice(j * NCH, (j + 1) * NCH)
                nc.sync.dma_start(out=w8sb[0][:, 0, sl], in_=w8[0:P, sl])
                nc.sync.dma_start(
                    out=w8sb[0][:, 1, sl],
                    in_=w8[0:P, NOUT + j * NCH:NOUT + (j + 1) * NCH],
                )
            if KTB:
                nc.scalar.dma_start(out=xmb_0[:, :, :], in_=xhb[0:P, :])
            for t in range(1, NPAIR // 2 + 1):
                nc.sync.dma_start(
                    out=w8sb[t][:, :, :], in_=w8[t * P:(t + 1) * P, :]
                )

            # prioritize the first half of the W stream on the rings: a
            # tiny SBUF->DRAM dma stalls the sync queue until pair 3
            # lands, so later tiles only hit the rings afterwards.
            wh_gate = dpool.tile([1, 1], f8, name="wh_gate")
            nc.sync.dma_start(out=wh_gate[:, :],
                              in_=w8sb[NPAIR // 2][0:1, 0, 0:1])
            for t in range(NPAIR // 2 + 1, NPAIR):
                nc.sync.dma_start(
                    out=w8sb[t][:, :, :], in_=w8[t * P:(t + 1) * P, :]
                )
            for i in range(KTB):
                nc.sync.dma_start(
                    out=wbsb[i][:, :], in_=wb[i * P:(i + 1) * P, :]
                )
            wlast = wbsb[KTB - 1] if KTB else w8sb[NPAIR - 1]

            # PE pre-warm on the first x slice: keeps the HAM activity
            # clock-gate open before real work; plain fp8 matmuls.
            pswarm = ppool.tile([P, NCH], f32, name="ps0", tag="ps0")
            for _ in range(22):
                nc.tensor.matmul(
                    pswarm[:, 0:P], lhsT=xm8_0[:, 0, :], rhs=xm8_0[:, 0, :],
                    start=True, stop=True,
                )

            def load_xm(m, eng):
                t8 = xpool.tile([P, KT8, P], f8, name="xm8", tag="xm8")
                eng.dma_start(out=t8[:, :, :], in_=xh8[m * P:(m + 1) * P, :])
                if not KTB:
                    return t8, None
                tb = xpool.tile([P, KTB, P], bf16, name="xmb", tag="xmb")
                eng.dma_start(out=tb[:, :, :], in_=xhb[m * P:(m + 1) * P, :])
                return t8, tb

            xm_pre = [(xm8_0, xmb_0), load_xm(1, nc.scalar)]

            # rendezvous the 8 cores while the weight DMAs stream in
            sync_in = dpool.tile([1, 1], f32, name="sync_in")
            sync_out = dpool.tile(
                [1, NCORES], f32, name="sync_out", addr_space="Shared"
            )
            nc.gpsimd.collective_compute(
                "AllGather",
                ALU.bypass,
                replica_groups=[list(range(NCORES))],
                ins=[sync_in.opt()],
                outs=[sync_out.opt()],
            )
            vsb = wpool.tile([P, NOUT], f32, name="vsb")
            nc.scalar.dma_start(out=vsb[:, :], in_=vr[:, :])
            # ones row for the tail's PE partition-broadcast
            ones_t = spool.tile([1, P], f32, name="ones_t")
            nc.vector.memset(ones_t[0:1, :], 1.0)

            # gate the early x prefetches behind W completion
            wgate = spool.tile([1, 1], bf16, name="wgate")
            nc.gpsimd.tensor_copy(wgate[0:1, 0:1], wlast[0:1, 0:1]
                                  if KTB else wlast[0:1, 0, 0:1])

            scores = spool.tile([P, MT], f32, name="scores")
            expv = spool.tile([P, MT], f32, name="expv")
            zrow = spool.tile([P, 1], f32, name="zrow")

            def alloc_work(m):
                psums = []
                for j in range(NCK):
                    ps = ppool.tile([P, NCH], f32, name=f"ps{j}", tag=f"ps{j}")
                    psums.append(ps)
                tmt = tpool.tile([P, NOUT], f32, name="tmt", tag="tmt")
                umt = tpool.tile([P, NOUT], f32, name="umt", tag="umt")
                acc = tpool.tile([P, NCK], f32, name="acc", tag="acc")
                return psums, tmt, umt, acc

            def mm_pair(psum, x8, t, j, start):
                nc.tensor.matmul(
                    psum[:, :],
                    lhsT=x8[:, 2 * t:2 * t + 2, :],
                    rhs=w8sb[t][:, :, j * NCH:(j + 1) * NCH],
                    start=start, stop=(not KTB and t == NPAIR - 1),
                    perf_mode=DR,
                )

            def mm_bf(psum, xb, i, j, stop):
                nc.tensor.matmul(
                    psum[:, :],
                    lhsT=xb[:, i, :],
                    rhs=wbsb[i][:, j * NCH:(j + 1) * NCH],
                    start=False, stop=stop,
                )

            def drain(m, psums, tmt, umt, acc, j):
                sl = slice(j * NCH, (j + 1) * NCH)
                nc.scalar.activation(tmt[:, sl], psums[j][:, :], AF.Tanh,
                                     scale=DESCALE)
                nc.vector.scalar_tensor_tensor(
                    out=umt[:, sl],
                    in0=tmt[:, sl],
                    scalar=1.0,
                    in1=vsb[:, sl],
                    op0=ALU.mult,
                    op1=ALU.mult,
                    accum_out=acc[:, j:j + 1],
                )

            def finish_scores(m, acc):
                nc.vector.tensor_reduce(
                    scores[:, m:m + 1], acc[:, :], AX.X, ALU.add
                )
                nc.scalar.activation(
                    expv[:, m:m + 1], scores[:, m:m + 1], AF.Exp
                )

            # tiles 0 and 1 interleaved over the W stream: 8 matmuls per
            # arriving pair-tile keep the PE saturated while W lands
            work01 = [alloc_work(0), alloc_work(1)]
            for t in range(NPAIR):
                for m in (0, 1):
                    for j in range(NCK):
                        mm_pair(work01[m][0][j], xm_pre[m][0], t, j,
                                start=(t == 0))
            for i in range(KTB):
                for m in (0, 1):
                    for j in range(NCK):
                        mm_bf(work01[m][0][j], xm_pre[m][1], i, j,
                              stop=(i == KTB - 1))
            for m in (0, 1):
                psums, tmt, umt, acc = work01[m]
                for j in range(NCK):
                    drain(m, psums, tmt, umt, acc, j)
                finish_scores(m, acc)

            for m in range(2, MT):
                if m < 10:
                    eng = nc.gpsimd
                else:
                    eng = nc.sync if m % 2 == 0 else nc.gpsimd
                xm8, xmb = load_xm(m, eng)
                psums, tmt, umt, acc = alloc_work(m)

                if m < MT - 1:
                    for t in range(NPAIR):
                        for j in range(NCK):
                            mm_pair(psums[j], xm8, t, j, start=(t == 0))
                    for i in range(KTB):
                        for j in range(NCK):
                            mm_bf(psums[j], xmb, i, j, stop=(i == KTB - 1))
                    for j in range(NCK):
                        drain(m, psums, tmt, umt, acc, j)
                    finish_scores(m, acc)
                else:
                    # last tile: chunk-major so each chunk drains while
                    # the next chunk's matmuls run, shrinking pieces
                    acc10 = tpool.tile(
                        [P, 2 * NCK + 1], f32, name="acc10", tag="acc10"
                    )
                    NH = NCH // 2
                    NQ = NCH // 4
                    ac = 0
                    for j in range(NCK):
                        for t in range(NPAIR):
                            mm_pair(psums[j], xm8, t, j, start=(t == 0))
                        for i in range(KTB):
                            mm_bf(psums[j], xmb, i, j, stop=(i == KTB - 1))
                        widths = [NH, NH] if j < NCK - 1 else [NH, NQ, NQ]
                        off = 0
                        for w in widths:
                            sl = slice(j * NCH + off, j * NCH + off + w)
                            psl = slice(off, off + w)
                            nc.scalar.activation(
                                tmt[:, sl], psums[j][:, psl], AF.Tanh,
                                scale=DESCALE,
                            )
                            nc.vector.scalar_tensor_tensor(
                                out=umt[:, sl],
                                in0=tmt[:, sl],
                                scalar=1.0,
                                in1=vsb[:, sl],
                                op0=ALU.mult,
                                op1=ALU.mult,
                                accum_out=acc10[:, ac:ac + 1],
                            )
                            off += w
                            ac += 1
                    nc.vector.tensor_reduce(
                        scores[:, m:m + 1], acc10[:, :], AX.X, ALU.add
                    )
                    nc.scalar.activation(
                        expv[:, m:m + 1], scores[:, m:m + 1], AF.Exp
                    )

            # ---- softmax over the global N via one AllGather ----
            from concourse import bass_isa

            nc.vector.tensor_reduce(
                zrow[:, 0:1], expv[:, :], AX.X, ALU.add
            )
            zloc = spool.tile([P, 1], f32, name="zloc")
            nc.gpsimd.partition_all_reduce(
                zloc[:, 0:1], zrow[:, 0:1], channels=P,
                reduce_op=bass_isa.ReduceOp.add,
            )
            zin = dpool.tile([1, 1], f32, name="zin")
            zout = dpool.tile(
                [1, NCORES], f32, name="zout", addr_space="Shared"
            )
            nc.gpsimd.dma_start(out=zin[:, :], in_=zloc[0:1, 0:1])
            nc.gpsimd.collective_compute(
                "AllGather",
                ALU.bypass,
                replica_groups=[list(range(NCORES))],
                ins=[zin.opt()],
                outs=[zout.opt()],
            )
            # read the 8 partials into one partition, reduce, then
            # broadcast to 128 partitions with a 1-row PE ones-matmul
            # (the stride-0 DRAM broadcast read cost ~6us in gpsimd
            # dge+drain time)
            zgb = spool.tile([1, NCORES], f32, name="zgb")
            nc.sync.dma_start(out=zgb[0:1, :], in_=zout[0:1, :])
            zs = spool.tile([1, 1], f32, name="zs")
            nc.vector.tensor_reduce(zs[0:1, 0:1], zgb[0:1, :], AX.X, ALU.add)
            psb = ppool.tile([P, NCH], f32, name="ps0", tag="ps0")
            nc.tensor.matmul(
                psb[:, 0:1], lhsT=ones_t[0:1, :], rhs=zs[0:1, 0:1],
                start=True, stop=True,
            )
            rzb = spool.tile([P, 1], f32, name="rzb")
            nc.vector.reciprocal(rzb[:, 0:1], psb[:, 0:1])
            outsb = spool.tile([P, MT], f32, name="outsb")
            MH = MT // 2
            nc.vector.tensor_scalar_mul(
                outsb[:, 0:MH], expv[:, 0:MH], rzb[:, 0:1]
            )
            nc.sync.dma_start(out=out_ext[:, 0:MH], in_=outsb[:, 0:MH])
            nc.vector.tensor_scalar_mul(
                outsb[:, MH:MT], expv[:, MH:MT], rzb[:, 0:1]
            )
            nc.scalar.dma_start(out=out_ext[:, MH:MT], in_=outsb[:, MH:MT])

    nc.finalize()
    _strip_redundant_ldweights(nc)
    return nc


def _strip_redundant_ldweights(nc):
    """Bacc emits one InstLdweights per matmul even when consecutive
    matmuls share the stationary operand. Drop the redundant ones."""
    def sig(arg):
        return (
            getattr(arg, "memref", None),
            getattr(arg, "offset", None),
            str(getattr(arg, "ap", None)),
        )

    removed = 0
    for bb in nc.main_func.blocks:
        keep = []
        last = None
        for inst in bb.instructions:
            if "Ldweights" in type(inst).__name__:
                s = sig(inst.ins[0])
                si = inst.sync_info
                if s == last and (
                    si is None or (not si.on_wait and not si.on_update)
                ):
                    removed += 1
                    continue
                last = s
            keep.append(inst)
        bb.instructions = keep
    return removed


# ---------------- host-side quantization ----------------

def _shaped_round(A, wcol, nscan):
    """e4m3 rounding of A (already scaled) with per-row flip choices so
    sum_j wcol[j]*eps[i,j] ~= 0, flips picked by damage/benefit greedy
    (near-boundary elements first) to keep eps energy ~unchanged.

    The alternative rounding (fp8 neighbor on the other side of A) is
    computed with sign-magnitude bit arithmetic on the e4m3 encoding."""
    A = np.clip(np.asarray(A, dtype=np.float32), -240.0, 240.0)
    q = A.astype(F8)
    qf = q.astype(np.float32)
    eps = qf - A
    u = q.view(np.uint8)
    sgn = u & np.uint8(0x80)
    mag = u & np.uint8(0x7F)
    pos = sgn == 0
    toward_pos = eps < 0          # q < A: the other neighbor is above q
    away = toward_pos == pos      # step increases |value|
    newmag = np.where(away, mag + np.uint8(1), mag - np.uint8(1))
    newsgn = sgn.copy()
    cross = (mag == 0) & ~away    # +/-0 stepping across zero
    newsgn = np.where(cross, sgn ^ np.uint8(0x80), newsgn)
    newmag = np.where(cross, np.uint8(1), newmag)
    alt = (newsgn | newmag).astype(np.uint8).view(F8).astype(np.float32)
    alt = np.where(eps == 0, qf, alt)
    epsa = alt - A
    w32 = wcol.astype(np.float32)
    c = (epsa - eps) * w32
    damage = (w32 ** 2) * (epsa ** 2 - eps ** 2)
    with np.errstate(divide="ignore", invalid="ignore"):
        ratio = np.where(np.abs(c) > 0, damage / np.abs(c), np.inf)
    nscan = min(nscan, A.shape[1] - 1)
    part = np.argpartition(ratio, nscan, axis=1)[:, :nscan]
    subr = np.take_along_axis(ratio, part, axis=1)
    subo = np.argsort(subr, axis=1)
    order = np.take_along_axis(part, subo, axis=1)
    c_s = np.take_along_axis(c, order, axis=1)
    g = (eps.astype(np.float64) @ wcol.astype(np.float64)).astype(np.float32)
    t = -g.copy()
    flip_s = np.zeros((A.shape[0], nscan), dtype=bool)
    for _ in range(2):
        for step in range(nscan):
            cn = np.where(flip_s[:, step], 0.0, c_s[:, step])
            take = np.abs(t - cn) < np.abs(t)
            t += np.where(take, -cn, 0.0)
            flip_s[:, step] |= take
    flip = np.zeros(A.shape, dtype=bool)
    np.put_along_axis(flip, order, flip_s, axis=1)
    return np.where(flip, alt, q).astype(F8)


def _prep_core_inputs(s, h, W, v):
    """Quantize + lay out per-core inputs."""
    x = np.concatenate([s, h], axis=1)                   # [N, KIN] f32
    Wt = np.ascontiguousarray(W.T)                       # [KIN, NOUT]
    vv = v.reshape(-1).astype(np.float64)
    hvec = Wt.astype(np.float64) @ vv                    # [KIN]

    W8 = _shaped_round(Wt[:KC].astype(np.float64) * SW, vv, nscan=512)
    X8 = _shaped_round(x[:, :KC].astype(np.float64) * SX, hvec[:KC],
                       nscan=384)

    # W pair tiles: w8[t*128+kk, i*NOUT+n] = W8[(2t+i)*128+kk, n]
    w8v = (
        W8.reshape(NPAIR, 2, P, NOUT)
        .transpose(0, 2, 1, 3)
        .reshape(NPAIR * P, 2 * NOUT)
    )
    w8v = np.ascontiguousarray(w8v)
    wbv = np.ascontiguousarray(Wt[KC:] * SB).astype(BF16)   # [256, NOUT]
    vrep = np.ascontiguousarray(
        np.broadcast_to(v.reshape(1, NOUT), (P, NOUT))
    ).astype(np.float32)

    xbv = (x[:, KC:] * SB).astype(BF16)                     # [N, 256]

    in_maps = []
    for c in range(NCORES):
        sl = slice(c * NS, (c + 1) * NS)
        x8c = X8[sl]                                        # [NS, KC]
        xh8 = (
            x8c.reshape(MT, P, KT8, P)
            .transpose(0, 3, 2, 1)
            .reshape(NS, KC)
        )
        xh8 = np.ascontiguousarray(xh8)
        xbc = xbv[sl]
        xhb = (
            xbc.reshape(MT, P, KTB, P)
            .transpose(0, 3, 2, 1)
            .reshape(NS, KTB * P)
        )
        xhb = np.ascontiguousarray(xhb)
        in_maps.append(
            {"xh8": xh8, "xhb": xhb, "w8": w8v, "wb": wbv, "vr": vrep}
        )
    return in_maps


_RUN_KW = {}  # test.py can inject trace=True etc.
LAST_RESULT = None


def kernel(s, h, W, v):
    from concourse.bass_utils import run_bass_kernel_spmd

    global LAST_RESULT
    s = np.asarray(s, dtype=np.float32)
    h = np.asarray(h, dtype=np.float32)
    W = np.asarray(W, dtype=np.float32)
    v = np.asarray(v, dtype=np.float32)

    in_maps = _prep_core_inputs(s, h, W, v)
    res = None
    for attempt in range(3):
        nc = _build_nc()
        try:
            res = run_bass_kernel_spmd(
                nc, in_maps, core_ids=list(range(NCORES)), **_RUN_KW
            )
            break
        except Exception:
            if attempt == 2:
                raise
            import time
            time.sleep(15)
    LAST_RESULT = res

    outs = []
    for c in range(NCORES):
        oc = np.asarray(res.results[c]["out"], dtype=np.float32)  # [P, MT]
        outs.append(oc.T.reshape(-1))
    return np.concatenate(outs).reshape(1, N).astype(np.float32)
